# revision 55
# baseline (speedup 1.0000x reference)
"""Trainium2 Bass kernel for CausalSE (chunked-mean-pool -> per-channel EMA ->
int4-fake-quant SE bottleneck -> sigmoid gate -> gated residual).

Contract: kernel(**inputs) takes FULL unsharded inputs (as produced by
setup_inputs) and returns the FULL [16, 2048, 4096] float32 output.
Internally shards batch 16 -> 8 NeuronCores (2 per core), replicating the
small weights, and runs a single-pass streaming Bass/Tile kernel per core.

Algorithm notes:
  - pooled mean and the (1-r) EMA input scale are folded into the first SE
    matmul weights: scan computes q[t] = r*q[t-1] + chunk_sum[t], and
    W1' = fq(w1) * ((1-r)/16) per input channel, so h = s*q never needs to be
    materialized.
  - EMA runs as one hardware TensorTensorScan per (batch, time-block) over the
    flattened (channel-block, pooled-t) axis; r is masked to 0 at each
    channel-block's first pooled step so segments don't leak, and the carry
    from the previous time block is injected into the first chunk-sum.
  - Weight fake-quant (int4 symmetric, round-half-even) is exact host-side
    preprocessing of tiny tensors; all x-dependent compute runs on device.

Performance notes (the kernel is HBM-DMA-bound; each choice below keeps the
other engines hidden underneath the 64 MiB/core of streaming traffic):
  - x and the output stream as bf16 (host casts): halves HBM bytes for
    ~3e-3 relative error, well inside the 2e-2 budget. SE matmul weights and
    activations are bf16 too (HW fp32 matmul is ~4x slower).
  - the host pre-permutes x to [batch, block, partition, chan-blk, t] so each
    512-step block is one fully contiguous 2 MiB DMA per direction.
  - chunk-pooling runs as a pairwise bf16 tensor_add tree (DVE tensor_reduce
    only has a 1x uop; all-step-1 bf16 tensor_tensor gets the 2x packed mode).
  - the sigmoid reads the whole block's PSUM bank through a stride-0 AP and
    writes the gate already expanded to chunk resolution, so the big gating
    multiply is step-1 on both operands (2x mode); the per-channel-block bias
    is applied by a rank-1 matmul on the otherwise idle PE.
  - xbufs=3 measured faster than deeper buffering (6-8) on hardware, and
    interleaving the two batch elements' blocks (b0k0, b1k0, b0k1, ...)
    doubles the slack on the EMA carry chain, hiding the SE-chain latency.
"""

import contextlib

import ml_dtypes
import numpy as np

import concourse.bacc as bacc
import concourse.mybir as mybir
import concourse.tile as tile
from concourse import bass_utils

F32 = mybir.dt.float32
BF16 = mybir.dt.bfloat16
NP_BF16 = ml_dtypes.bfloat16

B = 16
C = 2048
L = 4096
CHUNK = 16
HID = 256
QMAX = 7
EPS = 1e-5
N_CORES = 8
BPC = B // N_CORES          # batches per core = 2
P = 128
NCB = C // P                # channel blocks = 16
NOC = HID // P              # hidden (SE bottleneck) blocks = 2
TBLK = 512                  # time elements per streamed block
NTB = L // TBLK             # time blocks = 8
TP = TBLK // CHUNK          # pooled steps per block = 32
CONTIG = 1                  # host pre-permutes x so block DMAs are contiguous
GEXP = 1                    # expand gate to chunk resolution on ACT

_CACHE = {}


def _emit_body(nc, xd, outd, w1, w2, b1, b2, rmask, rlast,
               xpool, spool, carrypool, ps1, ps2, tblk=TBLK, ablate=(),
               contig=0, rdeng=0, muleng=0, treered=0, gexp=0,
               ones=None, b2row=None, interleave=0):
    """One full pass over this core's two batch elements.

    Emission is software-pipelined: each (b, k) iteration emits this block's
    load/pool/scan/SE-gate, but the gate-multiply + store of the PREVIOUS
    block. Engine instruction streams execute in order, so emitting mul(k)
    right after gate(k) would stall the whole DVE stream on the PE/ACT SE
    chain; delaying it one block keeps DVE busy with pooling while the tiny
    SE matmuls for the previous block finish on PE/ACT.
    """
    ntb = L // tblk
    tp = tblk // CHUNK

    pending = None  # (xt, gate, b, t0) awaiting mul+store

    def flush_pending():
        nonlocal pending
        if pending is None:
            return
        xt, gate, b, k = pending
        if "mul" not in ablate:
            x4 = xt[:].rearrange("p cb (tp ch) -> p cb tp ch", ch=CHUNK)
            if gexp:
                # gate already expanded to chunk resolution: both operands
                # step-1 bf16 -> DVE 2x packed mode
                nc.vector.tensor_mul(x4, x4, gate[:])
            else:
                gb = gate[:].unsqueeze(3).broadcast_to([P, NCB, tp, CHUNK])
                if muleng == 0:
                    nc.vector.tensor_mul(x4, x4, gb)
                elif muleng == 1:
                    nc.gpsimd.tensor_mul(x4, x4, gb)
                else:
                    h = NCB // 2
                    nc.vector.tensor_mul(x4[:, :h], x4[:, :h], gb[:, :h])
                    nc.gpsimd.tensor_mul(x4[:, h:], x4[:, h:], gb[:, h:])
        if contig:
            nc.scalar.dma_start(outd.ap()[b][k], xt[:])
        else:
            t0 = k * tblk
            nc.scalar.dma_start(
                outd.ap()[b][:, :, t0:t0 + tblk].transpose([1, 0, 2]),
                xt[:],
            )
        pending = None

    if interleave:
        sched = [(b, k) for k in range(ntb) for b in range(BPC)]
    else:
        sched = [(b, k) for b in range(BPC) for k in range(ntb)]
    qcs = []
    for b in range(BPC):
        qc_t = carrypool.tile([P, NCB], F32, tag=f"qc{b}")
        qcs.append(qc_t)
    if True:
        for b, k in sched:
            qc = qcs[b]
            xt = xpool.tile([P, NCB, tblk], BF16, tag="xt")
            if contig:
                nc.sync.dma_start(xt[:], xd.ap()[b][k])
            else:
                t0 = k * tblk
                nc.sync.dma_start(
                    xt[:],
                    xd.ap()[b][:, :, t0:t0 + tblk].transpose([1, 0, 2]),
                )
            x4 = xt[:].rearrange("p cb (tp ch) -> p cb tp ch", ch=CHUNK)

            sums = spool.tile([P, NCB, tp], F32, tag="sums")
            if "reduce" in ablate:
                nc.gpsimd.memset(sums[:], 0.01)
            elif treered:
                # pairwise-add tree: every level is all-bf16 step-1, so DVE
                # runs it in the 2x packed mode (tensor_reduce only has a 1x
                # uop and would cost ~2x more)
                tr = spool.tile([P, NCB, tp, 8], BF16, tag="tr")
                nc.vector.tensor_add(tr[:], x4[:, :, :, 0:8], x4[:, :, :, 8:16])
                nc.vector.tensor_add(tr[:, :, :, 0:4], tr[:, :, :, 0:4],
                                     tr[:, :, :, 4:8])
                nc.vector.tensor_add(tr[:, :, :, 0:2], tr[:, :, :, 0:2],
                                     tr[:, :, :, 2:4])
                nc.vector.tensor_add(sums[:], tr[:, :, :, 0], tr[:, :, :, 1])
            else:
                reng = nc.gpsimd if rdeng else nc.vector
                reng.reduce_sum(sums[:], x4, axis=mybir.AxisListType.X)

            if "se" in ablate:
                flush_pending()
                pending = (xt, sums, b, k)
                continue
            if k > 0:
                tmp = spool.tile([P, NCB], F32, tag="tmp")
                nc.vector.tensor_mul(tmp[:], qc[:], rlast[:])
                nc.vector.tensor_add(sums[:, :, 0], sums[:, :, 0], tmp[:])

            q = spool.tile([P, NCB, tp], BF16, tag="q")
            nc.vector.tensor_tensor_scan(
                q[:].rearrange("p cb tp -> p (cb tp)"),
                rmask[:].rearrange("p cb tp -> p (cb tp)"),
                sums[:].rearrange("p cb tp -> p (cb tp)"),
                initial=0.0,
                op0=mybir.AluOpType.mult,
                op1=mybir.AluOpType.add,
            )
            if k < ntb - 1:
                nc.vector.tensor_copy(qc[:], q[:, :, tp - 1])

            flush_pending()

            h1 = spool.tile([P, NOC, tp], BF16, tag="h1")
            for oc in range(NOC):
                acc = ps1.tile([P, tp], F32, tag="acc1")
                for cb in range(NCB):
                    nc.tensor.matmul(
                        acc[:],
                        w1[:, cb, oc * P:(oc + 1) * P],
                        q[:, cb, :],
                        start=(cb == 0),
                        stop=(cb == NCB - 1),
                    )
                nc.scalar.activation(
                    h1[:, oc, :], acc[:],
                    mybir.ActivationFunctionType.Relu,
                    bias=b1[:, oc:oc + 1],
                )

            if gexp:
                # all output blocks accumulate into one PSUM bank; per-block
                # bias lands via a 1-partition rank-1 matmul so a single
                # sigmoid (split in two for the PSUM 4K free-dim cap) can
                # write the gate already chunk-expanded for a 2x-mode mul
                acc2 = ps2.tile([P, NCB, tp], F32, tag="acc2big")
                for ob in range(NCB):
                    for kc in range(NOC):
                        nc.tensor.matmul(
                            acc2[:, ob, :],
                            w2[:, kc, ob * P:(ob + 1) * P],
                            h1[:, kc, :],
                            start=(kc == 0),
                            stop=False,
                        )
                    nc.tensor.matmul(
                        acc2[:, ob, :],
                        b2row[0:1, ob * P:(ob + 1) * P],
                        ones[0:1, :tp],
                        start=False,
                        stop=True,
                    )
                gate = spool.tile([P, NCB, tp, CHUNK], BF16, tag="gate16")
                if gexp == 2:
                    # sigmoid writes adjacent bf16 pairs; one int32-view copy
                    # replicates pairs to chunk width (half the elements)
                    g2 = spool.tile([P, NCB, tp, 2], BF16, tag="g2")
                    nc.scalar.activation(
                        g2[:], acc2[:].unsqueeze(3).broadcast_to(
                            [P, NCB, tp, 2]),
                        mybir.ActivationFunctionType.Sigmoid)
                    u32 = mybir.dt.uint32
                    nc.vector.tensor_copy(
                        gate[:].bitcast(u32),
                        g2[:].bitcast(u32).broadcast_to(
                            [P, NCB, tp, CHUNK // 2]),
                    )
                else:
                    gb = acc2[:].unsqueeze(3).broadcast_to(
                        [P, NCB, tp, CHUNK])
                    half = NCB // 2
                    nc.scalar.activation(
                        gate[:, :half], gb[:, :half],
                        mybir.ActivationFunctionType.Sigmoid)
                    nc.scalar.activation(
                        gate[:, half:], gb[:, half:],
                        mybir.ActivationFunctionType.Sigmoid)
            else:
                gate = spool.tile([P, NCB, tp], BF16, tag="gate")
                for ob in range(NCB):
                    acc2 = ps2.tile([P, tp], F32, tag="acc2")
                    for kc in range(NOC):
                        nc.tensor.matmul(
                            acc2[:],
                            w2[:, kc, ob * P:(ob + 1) * P],
                            h1[:, kc, :],
                            start=(kc == 0),
                            stop=(kc == NOC - 1),
                        )
                    nc.scalar.activation(
                        gate[:, ob, :], acc2[:],
                        mybir.ActivationFunctionType.Sigmoid,
                        bias=b2[:, ob:ob + 1],
                    )

            pending = (xt, gate, b, k)
    flush_pending()


def _build_module(repeat=1, tblk=TBLK, xbufs=3, sbufs=2, ps1b=2, ps2b=4, ablate=(),
                  contig=CONTIG, rdeng=0, muleng=0, treered=1, gexp=GEXP,
                  interleave=1):
    """Build the per-core module. repeat>1 wraps the body in a hardware loop
    that re-runs it (idempotently) for slope-based device timing."""
    tp = tblk // CHUNK
    ntb = L // tblk
    nc = bacc.Bacc("TRN2", target_bir_lowering=False, debug=False,
                   num_devices=N_CORES)

    xshape = [BPC, ntb, P, NCB, tblk] if contig else [BPC, NCB, P, L]
    xd = nc.dram_tensor("x", xshape, BF16, kind="ExternalInput")
    w1d = nc.dram_tensor("w1t", [P, NCB, HID], BF16, kind="ExternalInput")
    w2d = nc.dram_tensor("w2t", [P, NOC, C], BF16, kind="ExternalInput")
    b1d = nc.dram_tensor("b1t", [P, NOC], F32, kind="ExternalInput")
    b2d = nc.dram_tensor("b2t", [P, NCB], F32, kind="ExternalInput")
    rmd = nc.dram_tensor("rmask", [P, NCB, tp], F32, kind="ExternalInput")
    rld = nc.dram_tensor("rlast", [P, NCB], F32, kind="ExternalInput")
    b2rd = (nc.dram_tensor("b2r", [1, C], BF16, kind="ExternalInput")
            if gexp else None)
    outd = nc.dram_tensor("out", xshape, BF16, kind="ExternalOutput")

    with tile.TileContext(nc) as tc:
        with (
            tc.tile_pool(name="const", bufs=1) as cpool,
            tc.tile_pool(name="xp", bufs=xbufs) as xpool,
            tc.tile_pool(name="small", bufs=sbufs) as spool,
            tc.tile_pool(name="carry", bufs=1) as carrypool,
            tc.tile_pool(name="ps1", bufs=ps1b, space="PSUM") as ps1,
            tc.tile_pool(name="ps2", bufs=ps2b, space="PSUM") as ps2,
        ):
            w1 = cpool.tile([P, NCB, HID], BF16)
            w2 = cpool.tile([P, NOC, C], BF16)
            b1 = cpool.tile([P, NOC], F32)
            b2 = cpool.tile([P, NCB], F32)
            rmask = cpool.tile([P, NCB, tp], F32)
            rlast = cpool.tile([P, NCB], F32)
            nc.gpsimd.dma_start(w1[:], w1d.ap())
            nc.gpsimd.dma_start(w2[:], w2d.ap())
            nc.gpsimd.dma_start(b1[:], b1d.ap())
            nc.gpsimd.dma_start(b2[:], b2d.ap())
            nc.gpsimd.dma_start(rmask[:], rmd.ap())
            nc.gpsimd.dma_start(rlast[:], rld.ap())
            if gexp:
                b2row = cpool.tile([1, C], BF16)
                nc.gpsimd.dma_start(b2row[:], b2rd.ap())
                ones = cpool.tile([1, tp], BF16)
                nc.gpsimd.memset(ones[:], 1.0)
            else:
                b2row = ones = None

            rep = tc.For_i(0, repeat, 1) if repeat > 1 else contextlib.nullcontext()
            with rep:
                _emit_body(nc, xd, outd, w1, w2, b1, b2, rmask, rlast,
                           xpool, spool, carrypool, ps1, ps2, tblk=tblk,
                           ablate=ablate, contig=contig, rdeng=rdeng,
                           muleng=muleng, treered=treered, gexp=gexp,
                           ones=ones, b2row=b2row, interleave=interleave)

    nc.compile()
    return nc


def _fake_quant(w):
    w = np.asarray(w, np.float32)
    scale = (np.max(np.abs(w), axis=1, keepdims=True).astype(np.float32)
             / np.float32(QMAX) + np.float32(EPS)).astype(np.float32)
    wq = np.clip(np.round(w / scale), -QMAX, QMAX).astype(np.float32) * scale
    return wq.astype(np.float32)


def _host_prep(w1, b1, w2, b2, ema_r, tp=TP):
    r = np.asarray(ema_r, np.float32)
    s = ((np.float32(1.0) - r) / np.float32(CHUNK)).astype(np.float32)

    w1s = (_fake_quant(w1) * s[None, :]).astype(np.float32)        # [HID, C]
    w1t = np.ascontiguousarray(
        w1s.T.reshape(NCB, P, HID).transpose(1, 0, 2)).astype(NP_BF16)
    w2q = _fake_quant(w2)                                          # [C, HID]
    w2t = np.ascontiguousarray(
        w2q.T.reshape(NOC, P, C).transpose(1, 0, 2)).astype(NP_BF16)
    b1t = np.ascontiguousarray(np.asarray(b1, np.float32).reshape(NOC, P).T)
    b2t = np.ascontiguousarray(np.asarray(b2, np.float32).reshape(NCB, P).T)

    rpb = r.reshape(NCB, P).T                                      # [P, NCB]
    rmask = np.repeat(rpb[:, :, None], tp, axis=2).astype(np.float32)
    rmask[:, :, 0] = 0.0
    rlast = np.ascontiguousarray(rpb)
    return w1t, w2t, b1t, b2t, np.ascontiguousarray(rmask), rlast


def _make_in_maps(x, w1, b1, w2, b2, ema_r, tp=TP):
    w1t, w2t, b1t, b2t, rmask, rlast = _host_prep(w1, b1, w2, b2, ema_r, tp=tp)
    xh = np.asarray(x, np.float32).reshape(B, NCB, P, L).astype(NP_BF16)
    if CONTIG:
        tblk = tp * CHUNK
        xh = np.ascontiguousarray(
            xh.reshape(B, NCB, P, L // tblk, tblk).transpose(0, 3, 2, 1, 4))
    m = {"w1t": w1t, "w2t": w2t, "b1t": b1t, "b2t": b2t,
         "rmask": rmask, "rlast": rlast}
    if GEXP:
        m["b2r"] = np.asarray(b2, np.float32).reshape(1, C).astype(NP_BF16)
    return [dict(m, x=xh[c * BPC:(c + 1) * BPC]) for c in range(N_CORES)]


def kernel(x, w1, b1, w2, b2, ema_r):
    if "nc" not in _CACHE:
        _CACHE["nc"] = _build_module()
    nc = _CACHE["nc"]

    in_maps = _make_in_maps(x, w1, b1, w2, b2, ema_r)
    res = bass_utils.run_bass_kernel_spmd(nc, in_maps,
                                          core_ids=list(range(N_CORES)))
    if CONTIG:
        out = np.empty((B, NTB, P, NCB, TBLK), np.float32)
        for c in range(N_CORES):
            out[c * BPC:(c + 1) * BPC] = np.asarray(res.results[c]["out"])
        out = np.ascontiguousarray(out.transpose(0, 3, 2, 1, 4))
    else:
        out = np.empty((B, NCB, P, L), np.float32)
        for c in range(N_CORES):
            out[c * BPC:(c + 1) * BPC] = np.asarray(res.results[c]["out"])
    return out.reshape(B, C, L)

